# revision 83
# baseline (speedup 1.0000x reference)
"""Multi-head attention (B=8, N=1024, C=768, H=12) on 8 Trainium2 NeuronCores.

Sharding: data-parallel over batch — one batch element per core, no collectives.

Key structure (v2 — flipped attn@V + single-group fp8 compensation):
  - QKV in fp8-e4m3 with error compensation, but weights stored as 16*w with
    DIRECT e4m3 residuals (no 16x residual scaling): all 3 compensation
    passes (hi*hi + hi*lo + lo*hi) accumulate into ONE PSUM group, so the
    per-chunk DVE work is a single copy (no scalar_tensor_tensor folds).
    The global 16x cancels downstream: exp scale absorbs 1/256 from Q16*K16,
    and the ones-column used for row-sums is set to 16 so O'/rowsum = O.
  - attn@V flipped: P^T (bf16, written directly by the ACT exp) is the
    STATIONARY operand, V (bf16, 16x) the moving one. Output lands as
    [n=128 partitions, d=64 free] — full 128-partition utilization, halving
    attn@V PE cost vs the [65, 512] layout. Row-sums via 1-cycle ones-column
    matmuls into a shared PSUM misc tile.
  - Normalization becomes a per-partition scalar in the [n, d] layout:
    reciprocal of [128, 8] gathered row-sums + tensor_scalar multiplies
    (emitted as lazy DVE fillers to avoid head-end queue bursts).
    No DRAM-bounce stride-0 broadcast, no head-10/11 special cases.
  - [n, d] -> [d, n] via XBAR DMA transposes (SBUF->SBUF bf16) on the idle
    SP queue: no PE cycles, no PSUM, no vector-engine copies. Projection
    runs in bf16 (same PE cost as f32r, half the weight DMA); k0-2 and k3
    chunks stream as fillers with DVE adds; the k4/k5 tail folds the
    accumulated k0-3 back in with an f32r identity matmul so the final
    combine is a 3-engine copy + DMA, not an add chain.
  - PSUM: psA/psB [128,1024] S double-buffer (4 banks; also reused as
    startup/tail chunk slots), two O' head tiles [128,512] (2), one filler
    tile with ping-pong 256-col halves (1), one rowsum tile (1) = 8 banks.
    start=True zeroes a whole 2KB bank, so each bank gets exactly one
    start per accumulation round (O': j0/nt0 only; rowsums: once ever),
    and only single-shot groups share banks with in-flight data.
  - GPSIMD cannot touch PSUM: Pool does only DMA issue; all PSUM reads go
    through DVE/ACT.
  - PE is the critical engine: ~114us busy (vs 135us for the unflipped
    baseline); exp stream on ACT ~100us right behind it. Cost-model span
    131.6us vs 154.9us baseline.
"""

import numpy as np

_STATE = {}

B, N, C = 8, 1024, 768
H, D = 12, 64
KT = 6           # contraction tiles of 128 over C
P = 128
NT = N // P      # 8 n-tiles
PAIRS = H // 2   # 6 head pairs
VC = 3           # V weight chunks (4 heads each)


def _patch_tile_drain():
    """This walrus build rejects >1 sem wait on a CTRL (Drain) instruction.

    TileContext's exit puts one wait per outstanding semaphore on the final SP
    Drain; redistribute them across single-wait NOPs preceding the drain.
    """
    import bass_rust
    import concourse.tile as tile
    from concourse.vector_clock import ScopedClock

    if getattr(tile.TileContext, "_ant_drain_patched", False):
        return

    SyncInfo = bass_rust.SyncInfo

    def _drain_and_barrier(self, tick_clock, wait_clock):
        nc = self.nc
        probe = nc.sync.nop(nofuse=True)
        wait_clock.add_sem_waits(
            probe.ins, ScopedClock({None: tick_clock.global_clock})
        )
        si = probe.ins.sync_info
        waits = list(si.on_wait or []) if si is not None else []
        updates = list(si.on_update or []) if si is not None else []
        if len(waits) > 1:
            probe.ins.sync_info = SyncInfo(on_wait=waits[:1], on_update=updates)
            for w in waits[1:]:
                extra = nc.sync.nop(nofuse=True)
                extra.ins.sync_info = SyncInfo(on_wait=[w], on_update=[])
        nc.sync.drain()

        nc.all_engine_barrier()
        assert self.sems is not None
        popped = nc._tile_sem_poison_stack.pop()
        assert popped is self._sem_poison
        nc.clear_and_free_semaphores(list(self.sems.allocated().values()))
        nc.all_engine_barrier()

    tile.TileContext._drain_and_barrier = _drain_and_barrier
    tile.TileContext._ant_drain_patched = True


def _split_multi_waits(nc):
    """This walrus build allows at most ONE sem wait per instruction.

    Tile's wait assignment routinely puts several; hoist all but the last onto
    single-wait NOPs inserted immediately before the instruction on the same
    engine (engines execute block instructions in order, so semantics are
    unchanged).
    """
    from concourse import mybir

    for fn in nc.m.functions:
        for bb in fn.blocks:
            out, changed = [], False
            for inst in bb.instructions:
                si = inst.sync_info
                waits = list(si.on_wait) if (si is not None and si.on_wait) else []
                if len(waits) > 1:
                    changed = True
                    for w in waits[:-1]:
                        nop = mybir.InstNoOp(
                            name=f"I-ws{nc.next_id()}",
                            engine=inst.engine,
                            bass_nofuse=True,
                            sync_info=mybir.SyncInfo(on_wait=[w], on_update=[]),
                        )
                        nc.register_instruction(nop)
                        out.append(nop)
                    inst.sync_info = mybir.SyncInfo(
                        on_wait=[waits[-1]], on_update=list(si.on_update or [])
                    )
                out.append(inst)
            if changed:
                bb.instructions = out


def _build_nc(trace_sim=False):
    from contextlib import ExitStack

    import concourse.bass as bass
    import concourse.tile as tile
    from concourse import mybir

    _patch_tile_drain()

    f32 = mybir.dt.float32
    f32r = mybir.dt.float32r
    bf16 = mybir.dt.bfloat16
    f8 = mybir.dt.float8e4

    nc = bass.Bass("TRN2", target_bir_lowering=False, debug=False, num_devices=1)

    xT = nc.dram_tensor("xT", [KT, P, 2, N], f8, kind="ExternalInput").ap()
    wqk = nc.dram_tensor("wqk", [PAIRS, P, 2 * 3 * 2 * 256], f8,
                         kind="ExternalInput").ap()
    wv = nc.dram_tensor("wv", [VC, P, 2 * 3 * 2 * 256], f8,
                        kind="ExternalInput").ap()
    ptw = nc.dram_tensor("ptw", [P, KT, C], bf16, kind="ExternalInput").ap()
    bias = nc.dram_tensor("bias", [P, C], f32, kind="ExternalInput").ap()
    ident = nc.dram_tensor("ident", [P, P], bf16, kind="ExternalInput").ap()
    ident32 = nc.dram_tensor("ident32", [P, P], f32r, kind="ExternalInput").ap()
    y = nc.dram_tensor("y", [N, C], f32, kind="ExternalOutput").ap()

    Exp = mybir.ActivationFunctionType.Exp
    DR = mybir.MatmulPerfMode.DoubleRow
    SCALE = float(D) ** -0.5 / 256.0   # /256 cancels the 16x on Q and K

    with tile.TileContext(nc, trace_sim=trace_sim) as tc, ExitStack() as ctx:
        kilo = ctx.enter_context(tc.tile_pool(name="kilo", bufs=1))     # x8
        wqkp = ctx.enter_context(tc.tile_pool(name="wqk", bufs=2))
        wvp = ctx.enter_context(tc.tile_pool(name="wv", bufs=3))
        qkp = ctx.enter_context(tc.tile_pool(name="qk", bufs=4))
        vp = ctx.enter_context(tc.tile_pool(name="v", bufs=18))
        ptp = ctx.enter_context(tc.tile_pool(name="pt", bufs=4))
        normp = ctx.enter_context(tc.tile_pool(name="nrm", bufs=18))
        rcpp = ctx.enter_context(tc.tile_pool(name="rcp", bufs=3))
        osbp = ctx.enter_context(tc.tile_pool(name="osb", bufs=6))
        outp = ctx.enter_context(tc.tile_pool(name="out", bufs=8))
        onep = ctx.enter_context(tc.tile_pool(name="one", bufs=1))
        psA = ctx.enter_context(tc.tile_pool(name="psA", bufs=1, space="PSUM"))
        psB = ctx.enter_context(tc.tile_pool(name="psB", bufs=1, space="PSUM"))
        poA = ctx.enter_context(tc.tile_pool(name="poA", bufs=1, space="PSUM"))
        poB = ctx.enter_context(tc.tile_pool(name="poB", bufs=1, space="PSUM"))
        fillp = ctx.enter_context(tc.tile_pool(name="fil", bufs=1, space="PSUM"))
        miscp = ctx.enter_context(tc.tile_pool(name="msc", bufs=1, space="PSUM"))

        # ---- persistent PSUM tiles (column-slice accumulation groups) ----
        # NOTE: a matmul with start=True marks the whole 2KB bank as
        # pending-zero, wiping any OTHER accumulation group in that bank.
        # So: misct holds ONLY the 96 one-col rowsum groups (one start ever);
        # transposes ride the fillt rotation (single-shot groups are safe:
        # completed data is read from plain memory by DVE/ACT).
        fillt = fillp.tile([P, 512], f32, tag="fill", name="fillt")
        misct = miscp.tile([P, 512], f32, tag="misc", name="misct")

        # ---- input DMAs ----
        x8 = kilo.tile([P, KT, 2, N], f8, tag="kilo", name="x8")
        wq_tiles = {}

        def prefetch_wq(t, eng):
            if t not in wq_tiles:
                wq_t = wqkp.tile([P, 2, 3, 2, 256], f8, tag="wqk", name=f"wq_{t}")
                eng.dma_start(wq_t[:], wqk[t])
                wq_tiles[t] = wq_t

        prefetch_wq(0, nc.gpsimd)  # leads the Pool queue: ready ~1.2us
        for k in range(KT):
            eng = (nc.sync, nc.scalar, nc.sync, nc.scalar,
                   nc.sync, nc.scalar)[k]
            eng.dma_start(x8[:, k, :, :], xT[k])

        # warm the ACT exp table set while input DMAs run
        warm = onep.tile([1, 4], f32)
        nc.vector.memset(warm[:], 0.0)
        warm2 = onep.tile([1, 4], f32)
        nc.scalar.activation(warm2[:], warm[:], Exp)

        # V weights chunks 0-1 early (Pool queue); 2 later
        wv_sb = [wvp.tile([P, 2, 3, 2, 256], f8, tag="wv", name=f"wvc_{c}")
                 for c in range(VC)]
        nc.gpsimd.dma_start(wv_sb[0][:], wv[0])
        nc.gpsimd.dma_start(wv_sb[1][:], wv[1])

        ident_sb = onep.tile([P, P], bf16)
        nc.sync.dma_start(ident_sb[:], ident[:])
        ident32_sb = onep.tile([P, P], f32r)
        nc.sync.dma_start(ident32_sb[:], ident32[:])
        bias_sb = onep.tile([P, C], f32)

        # ones column (=16, matching the 16x-scaled V) for row-sum matmuls
        ones_bf = onep.tile([P, 1], bf16)
        nc.vector.memset(ones_bf[:], 16.0)

        # PE p-state pre-warm: dummy matmuls while the first inputs stream in
        dm_sb = onep.tile([P, 128], bf16)
        nc.vector.memset(dm_sb[:], 0.0)
        for i in range(2):
            nc.tensor.matmul(fillt[0:1, 0:128], dm_sb[:, 0:1], dm_sb[:, 0:128],
                             start=True, stop=True, skip_group_check=True)

        # ---- QK chunks: 9 DR matmuls -> ONE psum group -> one DVE copy ----
        qt_sb, kt_sb = {}, {}
        qk_done = {}
        v_done = {}
        fill_idx = [0]

        def fill_half():
            h = fill_idx[0] % 2
            fill_idx[0] += 1
            return fillt[:, h * 256:(h + 1) * 256]

        def qk_chunk(t, which, ns, slot=None):
            store = qt_sb if which == 0 else kt_sb
            if t not in store:
                store[t] = qkp.tile([P, N], f32r, tag="qk",
                                    name=f"{'q' if which == 0 else 'k'}_{t}")
            wq_t = wq_tiles[t]
            if slot is None:
                slot = fill_half()
            ncol = slice(ns * 256, (ns + 1) * 256)
            wcol = slice(which * P, (which + 1) * P)
            passes = [(0, 0), (1, 0), (0, 1)]   # (w hi/lo, x hi/lo)
            i = 0
            for whl, xhl in passes:
                for kp in range(3):
                    nc.tensor.matmul(
                        slot, wq_t[:, whl, kp, :, wcol],
                        x8[:, 2 * kp: 2 * kp + 2, xhl, ncol],
                        start=(i == 0), stop=(i == 8), perf_mode=DR,
                        skip_group_check=True,
                    )
                    i += 1
            dest = store[t][:, ns * 256:(ns + 1) * 256]
            with nc.allow_low_precision(reason="f32r is f32 bits"):
                nc.vector.tensor_copy(dest, slot)
            qk_done[t] = qk_done.get(t, 0) + 1

        # ---- V chunks: [128 n, 256] (4 heads x 64), bf16, 16x scaled ----
        v_sb = {}

        def v_chunk(c, j):
            jcol = slice(j * P, (j + 1) * P)
            slot = fill_half()
            passes = [(0, 0), (0, 1), (1, 0)]   # (x hi/lo, w hi/lo)
            i = 0
            for xhl, whl in passes:
                for kp in range(3):
                    nc.tensor.matmul(
                        slot, x8[:, 2 * kp: 2 * kp + 2, xhl, jcol],
                        wv_sb[c][:, whl, kp, :, :],
                        start=(i == 0), stop=(i == 8), perf_mode=DR,
                        skip_group_check=True,
                    )
                    i += 1
            vt = vp.tile([P, 256], bf16, tag="v", name=f"v_{c}_{j}")
            with nc.allow_low_precision(reason="attn probs tolerate bf16 V"):
                nc.vector.tensor_copy(vt[:], slot)
            v_sb[(c, j)] = vt
            v_done[c] = v_done.get(c, 0) + 1

        # pair-0 QK immediately (chasing the input DMA arrivals). The S/O'
        # psum banks are idle at startup: give every chunk its own slot so
        # the chunks stream without ping-pong WAR stalls. S_0 needs all four
        # q chunks but only k chunk 0 — k chunks 1-3 are deferred into the
        # first head's j-loop so the exp stream starts ~3us earlier.
        st_a = psA.tile([P, N], f32, tag="psA", name="st_a")
        st_b = psB.tile([P, N], f32, tag="psB", name="st_b")
        st_slots = [st_a[:, i * 256:(i + 1) * 256] for i in range(4)] + \
                   [st_b[:, i * 256:(i + 1) * 256] for i in range(4)]
        for ns in range(4):
            qk_chunk(0, 0, ns, slot=st_slots[ns])
        for ns in range(4):
            qk_chunk(0, 1, ns, slot=st_slots[4 + ns])

        # ---- filler queue: PE work interleaved into the attention stream.
        # Priority: qk (gates the next pair's S) > v > transposes > proj.
        import heapq

        fillers = []
        fseq = [0]
        steps = [0]
        TOT_STEPS = 96

        def push_filler(prio, fn):
            heapq.heappush(fillers, (prio, fseq[0], fn))
            fseq[0] += 1

        def pop_filler(budget):
            for _ in range(budget):
                if fillers:
                    heapq.heappop(fillers)[2]()

        dve_fillers = []

        def step_fillers():
            steps[0] += 1
            left = max(1, TOT_STEPS - 8 - steps[0])
            budget = min(4, max(2, -(-len(fillers) // left)))
            pop_filler(budget)
            for _ in range(2):
                if dve_fillers:
                    dve_fillers.pop(0)()

        def ensure_qk(tp):
            # all 8 qk chunks of pair tp must be EMITTED before its first S
            while qk_done.get(tp, 0) < 8 and fillers:
                heapq.heappop(fillers)[2]()

        def ensure_v(c):
            while v_done.get(c, 0) < 8 and fillers:
                heapq.heappop(fillers)[2]()

        # ---- projection ----
        # k0-2: 256-col chunks + bias add into acc (pairs 3-4).
        # k3-4: 128-col chunks with a 4-quarter fill rotation + adds split
        #       DVE/Pool (pair 5 — add-latency-bound, so minimize WAR depth).
        # k5:   tail chunks fold acc back in via an identity matmul (f32r),
        #       then a 3-way-engine copy (ACT is free post-stream) + y DMA.
        ptw_sb = [None]
        acc_sb = {}
        q_idx = [0]
        cp_idx = [0]

        def get_acc(nt):
            if nt not in acc_sb:
                acc_sb[nt] = outp.tile([P, C], f32r, tag="out", name=f"acc_{nt}")
            return acc_sb[nt]

        def proj_g0(nt, cc):
            c0, w = cc * 256, 256
            slot = fill_half()
            for k in (0, 1, 2):
                nc.tensor.matmul(
                    slot,
                    o_sb[k][:, nt * P:(nt + 1) * P],
                    ptw_sb[0][:, k, c0:c0 + w],
                    start=(k == 0), stop=(k == 2),
                    skip_group_check=True,
                )
            dst = get_acc(nt)[:, c0:c0 + w]
            with nc.allow_low_precision(reason="f32r is f32 bits"):
                nc.vector.tensor_add(dst, slot, bias_sb[:, c0:c0 + w])

        def proj_k1(nt, cc, k):
            c0, w = cc * 256, 256
            slot = fill_half()
            nc.tensor.matmul(
                slot,
                o_sb[k][:, nt * P:(nt + 1) * P],
                ptw_sb[0][:, k, c0:c0 + w],
                start=True, stop=True,
                skip_group_check=True,
            )
            dst = get_acc(nt)[:, c0:c0 + w]
            with nc.allow_low_precision(reason="f32r is f32 bits"):
                nc.vector.tensor_add(dst, dst, slot)

        tail_slots = [None]

        def proj_tail(nt, cc):
            c0, w = cc * 256, 256
            slots = tail_slots[0]
            slot = slots[cp_idx[0] % len(slots)]
            cp_idx[0] += 1
            acc = get_acc(nt)
            for k in (4, 5):
                nc.tensor.matmul(
                    slot, o_sb[k][:, nt * P:(nt + 1) * P],
                    ptw_sb[0][:, k, c0:c0 + w],
                    start=(k == 4), stop=False, skip_group_check=True,
                )
            nc.tensor.matmul(
                slot, ident32_sb[:],
                acc[:, c0:c0 + w],
                start=False, stop=True, skip_group_check=True,
            )
            eng = (nc.scalar, nc.vector)[cp_idx[0] % 2]
            with nc.allow_low_precision(reason="f32r is f32 bits"):
                if eng is nc.scalar:
                    eng.copy(acc[:, c0:c0 + w], slot)
                else:
                    eng.tensor_copy(acc[:, c0:c0 + w], slot)
            # per-cc DMAs on the two idle HWDGE queues keep the final
            # copy->DMA chain short (Pool SWDGE DMAs cost ~1us engine time)
            deng = nc.sync if (nt + cc) % 2 == 0 else nc.scalar
            deng.dma_start(y[nt * P:(nt + 1) * P, c0:c0 + w],
                           acc[:, c0:c0 + w].bitcast(f32))

        # ---- attention ----
        o_sb = []          # per pair: [128 d-pair, N] bf16 (proj stationary)
        norm_pair = {}     # pair -> [128 n, 8*(2*64)] bf16 (normalized O)
        tr_idx = [0]

        def emit_S(t, head, j):
            hb = head * D
            pool, tg = (psA, "psA") if j % 2 == 0 else (psB, "psB")
            s_slot = pool.tile([P, N], f32, tag=tg, name=f"s_{2*t+head}_{j}")
            for ns in range(2):
                nc.tensor.matmul(
                    s_slot[:, ns * 512:(ns + 1) * 512],
                    kt_sb[t][hb:hb + D, j * P:(j + 1) * P],
                    qt_sb[t][hb:hb + D, ns * 512:(ns + 1) * 512],
                    start=True, stop=True,
                )
            return s_slot

        norms_done = {}

        def transpose_chunk(t, np2):
            # [n, d-pair] -> [d-pair, n] via XBAR DMA transposes on the idle
            # SP queue: no PE cycles, no PSUM, no DVE copies.
            while norms_done.get(t, 0) < 16 and dve_fillers:
                dve_fillers.pop(0)()
            for i in range(2):
                nt = 2 * np2 + i
                nc.sync.dma_start_transpose(
                    o_sb[t][:, nt * P:(nt + 1) * P],
                    norm_pair[t][:, nt * P:(nt + 1) * P])

        pre_s = [[]]
        rs_started = [False]
        for t in range(PAIRS):
            ot = osbp.tile([P, N], bf16, tag="osb", name=f"ot_{t}")
            o_sb.append(ot)
            if t < PAIRS - 1:
                prefetch_wq(t + 1, nc.gpsimd if t == 0 else nc.sync)
                for which in range(2):
                    for ns in range(4):
                        push_filler(
                            0, lambda t=t, w=which, n=ns: qk_chunk(t + 1, w, n))
            # v chunks: all of c=1 at pair 0; c=2 split over pairs 2-3
            v_push = {0: (1, range(NT)), 2: (2, range(0, 4)),
                      3: (2, range(4, NT))}
            if t in v_push:
                c, js = v_push[t]
                for j in js:
                    push_filler(1, lambda c=c, j=j: v_chunk(c, j))
            if t == 0:
                nc.sync.dma_start(wv_sb[2][:], wv[2])
            if t == 1:
                ptw_t = onep.tile([P, KT, C], bf16, name="ptw_sb")
                nc.gpsimd.dma_start(ptw_t[:], ptw[:])
                ptw_sb[0] = ptw_t
                nc.sync.dma_start(bias_sb[:], bias[:])
            if 1 <= t <= 5:
                # transposes for pair t-1 (norm tiles ready at end of pair t-1)
                for np2 in range(NT // 2):
                    push_filler(2, lambda tt=t - 1, np2=np2:
                                transpose_chunk(tt, np2))
            if t == 3:
                # k0-2 proj (pairs 0-2 transposed by pair 3's tr batch);
                # nt 0-3 here, nt 4-7 at pair 4
                for nt in range(4):
                    for cc in range(3):
                        push_filler(3, lambda nt=nt, cc=cc: proj_g0(nt, cc))
            if t == 4:
                for nt in range(4, NT):
                    for cc in range(3):
                        push_filler(3, lambda nt=nt, cc=cc: proj_g0(nt, cc))
                for nt in range(NT):
                    for cc in range(3):
                        push_filler(4, lambda nt=nt, cc=cc: proj_k1(nt, cc, 3))

            if t > 0:
                ensure_v(t // 2)
            for head in range(2):
                h = 2 * t + head
                opool, otag = (poA, "poA") if head == 0 else (poB, "poB")
                o_slot = opool.tile([P, 512], f32, tag=otag, name=f"o_{h}")
                vc = t // 2
                vb = h - 4 * vc  # head index within the 4-head V chunk
                nxt = (t + (head + 1) // 2, 1 - head) if h < 11 else None

                def emit_exp(j, s_slot):
                    # emitted immediately after its S matmuls so the exp's
                    # PE-clock wait covers ONLY the S
                    pt_t = ptp.tile([P, N], bf16, tag="pt", name=f"p_{h}_{j}")
                    nc.scalar.activation(pt_t[:], s_slot[:], Exp, scale=SCALE)
                    return pt_t

                if pre_s[0]:
                    s0, s1 = pre_s[0]
                else:
                    s0, s1 = emit_S(t, head, 0), emit_S(t, head, 1)
                pre_s[0] = []
                pts = [emit_exp(0, s0), emit_exp(1, s1)]
                if t == 0 and head == 0:
                    v_chunk(0, 0)
                    v_chunk(0, 1)
                for j in range(NT):
                    if j + 2 < NT:
                        pts.append(emit_exp(j + 2, emit_S(t, head, j + 2)))
                    elif nxt is not None:
                        # pre-emit the next head's S_0/S_1 so its exp stream
                        # starts without a boundary stall
                        ensure_qk(nxt[0])
                        pre_s[0].append(emit_S(nxt[0], nxt[1], j - 6))
                    vtile = v_sb[(vc, j)]
                    for nt in range(NT):
                        # start=True zeroes the whole bank: exactly ONE start
                        # per head (j0/nt0); later nt's j0 writes land on the
                        # pending-zero region = fresh accumulation
                        nc.tensor.matmul(
                            o_slot[:, nt * 64:(nt + 1) * 64],
                            pts[j][:, nt * P:(nt + 1) * P],
                            vtile[:, vb * D:(vb + 1) * D],
                            start=(j == 0 and nt == 0), stop=(j == NT - 1),
                            skip_group_check=True,
                        )
                        # rowsums share one bank across ALL heads: start once
                        nc.tensor.matmul(
                            misct[:, h * 8 + nt: h * 8 + nt + 1],
                            pts[j][:, nt * P:(nt + 1) * P],
                            ones_bf[:],
                            start=(not rs_started[0]), stop=(j == NT - 1),
                            skip_group_check=True,
                        )
                        rs_started[0] = True
                    if t == 0 and head == 0:
                        if j + 2 < NT:
                            v_chunk(0, j + 2)
                        steps[0] += 1
                    else:
                        step_fillers()
                # head end: reciprocal of the 8 gathered row-sums, then
                # per-partition normalize into the pair's norm tiles
                rc_t = rcpp.tile([P, NT], f32, tag="rcp", name=f"rc_{h}")
                nc.vector.reciprocal(rc_t[:], misct[:, h * 8:(h + 1) * 8])

                if t not in norm_pair:
                    norm_pair[t] = normp.tile(
                        [P, N], bf16, tag="nrm", name=f"nm_{t}")

                def norm_one(nt, t=t, head=head, o_slot=o_slot, rc_t=rc_t,
                             eng=None):
                    dst = norm_pair[t][:, nt * P + head * D:
                                       nt * P + (head + 1) * D]
                    src = o_slot[:, nt * D:(nt + 1) * D]
                    with nc.allow_low_precision(reason="bf16 attn out"):
                        if eng is nc.scalar:
                            nc.scalar.mul(dst, src, rc_t[:, nt:nt + 1])
                        else:
                            nc.vector.tensor_scalar_mul(
                                dst, src, rc_t[:, nt:nt + 1])
                    norms_done[t] = norms_done.get(t, 0) + 1

                if h < 11:
                    # lazily popped: avoids an 8-op DVE burst that blocks
                    # the chunk-copy stream
                    for nt in range(NT):
                        dve_fillers.append(lambda nt=nt, f=norm_one: f(nt))
                else:
                    norm11 = norm_one

        # ---- tail: pair-5 transposes + k4/k5 projection + output. The S
        # psum banks are free now: deep slot rotation so nothing ping-pong
        # stalls; leftover fillers interleave to hide the per-nt chain.
        st2a = psA.tile([P, N], f32, tag="psA", name="st2a")
        st2b = psB.tile([P, N], f32, tag="psB", name="st2b")
        tail_slots[0] = (
            [st2a[:, i * 256:(i + 1) * 256] for i in range(4)]
            + [st2b[:, i * 256:(i + 1) * 256] for i in range(4)]
        )
        while dve_fillers:
            dve_fillers.pop(0)()
        for np2 in range(NT // 2):
            # h11 norms pipelined per-np2, split ACT/DVE (both free now)
            norm11(2 * np2, eng=nc.scalar)
            norm11(2 * np2 + 1, eng=nc.vector)
            transpose_chunk(5, np2)
            for nt in (2 * np2, 2 * np2 + 1):
                for cc in range(3):
                    proj_tail(nt, cc)
            pop_filler(4)
        pop_filler(len(fillers))

    _split_multi_waits(nc)
    return nc


def _prep_shared(qkv_w, proj_w, proj_b):
    import ml_dtypes

    E4 = ml_dtypes.float8_e4m3fn
    BF = ml_dtypes.bfloat16
    f = np.float32

    def dsplit(a):
        hi = a.astype(E4)
        lo = (a - hi.astype(f)).astype(E4)
        return hi, lo

    wqT = np.ascontiguousarray(qkv_w[0:C].T.astype(f)) * 16.0      # [c, o]
    wkT = np.ascontiguousarray(qkv_w[C:2 * C].T.astype(f)) * 16.0
    wvT = np.ascontiguousarray(qkv_w[2 * C:3 * C].T.astype(f)) * 16.0

    wqk = np.zeros((PAIRS, P, 2, 3, 2, 256), E4)
    for t in range(PAIRS):
        for kp in range(3):
            for pe in range(2):  # k-tile within the DoubleRow pair
                k = 2 * kp + pe
                qh, ql = dsplit(wqT[k * P:(k + 1) * P, t * P:(t + 1) * P])
                kh, kl = dsplit(wkT[k * P:(k + 1) * P, t * P:(t + 1) * P])
                wqk[t, :, 0, kp, pe, 0:P] = qh
                wqk[t, :, 1, kp, pe, 0:P] = ql
                wqk[t, :, 0, kp, pe, P:256] = kh
                wqk[t, :, 1, kp, pe, P:256] = kl
    wqk = wqk.reshape(PAIRS, P, 2 * 3 * 2 * 256)

    wvh = np.zeros((VC, P, 2, 3, 2, 256), E4)
    for c in range(VC):
        for kp in range(3):
            for pe in range(2):
                k = 2 * kp + pe
                vh, vl = dsplit(
                    wvT[k * P:(k + 1) * P, 4 * c * D:(4 * c + 4) * D])
                wvh[c, :, 0, kp, pe, :] = vh
                wvh[c, :, 1, kp, pe, :] = vl
    wvh = wvh.reshape(VC, P, 2 * 3 * 2 * 256)

    ptw = proj_w.T.astype(f).reshape(KT, P, C).transpose(1, 0, 2)
    ptw = np.ascontiguousarray(ptw).astype(BF)
    bias_h = np.ascontiguousarray(np.broadcast_to(proj_b.astype(f), (P, C)))
    ident = np.eye(P, dtype=BF)
    ident32 = np.eye(P, dtype=f)
    return wqk, wvh, ptw, bias_h, ident, ident32


def kernel(x, qkv_w, proj_w, proj_b):
    import ml_dtypes
    from concourse.bass_utils import run_bass_kernel_spmd

    E4 = ml_dtypes.float8_e4m3fn
    x = np.asarray(x, np.float32)
    wqk, wvh, ptw, bias_h, ident, ident32 = _prep_shared(
        np.asarray(qkv_w), np.asarray(proj_w), np.asarray(proj_b)
    )

    if "nc" not in _STATE:
        _STATE["nc"] = _build_nc()
    nc = _STATE["nc"]

    in_maps = []
    for b in range(B):
        xf = np.ascontiguousarray(x[b].T).reshape(KT, P, N)
        xh = xf.astype(E4)
        xl = (xf - xh.astype(np.float32)).astype(E4)
        xTb = np.stack([xh, xl], axis=2)  # [KT, P, 2, N] fp8
        in_maps.append(
            {"xT": xTb, "wqk": wqk, "wv": wvh, "ptw": ptw, "bias": bias_h,
             "ident": ident, "ident32": ident32}
        )

    res = run_bass_kernel_spmd(nc, in_maps, core_ids=list(range(B)))
    return np.stack([res.results[b]["y"] for b in range(B)], axis=0)


# revision 97
# speedup vs baseline: 1.0156x; 1.0156x over previous
"""Multi-head attention (B=8, N=1024, C=768, H=12) on 8 Trainium2 NeuronCores.

Sharding: data-parallel over batch — one batch element per core, no collectives.

Key structure (v2 — flipped attn@V + single-group fp8 compensation):
  - QKV in fp8-e4m3 with error compensation, but weights stored as 16*w with
    DIRECT e4m3 residuals (no 16x residual scaling): all 3 compensation
    passes (hi*hi + hi*lo + lo*hi) accumulate into ONE PSUM group, so the
    per-chunk DVE work is a single copy (no scalar_tensor_tensor folds).
    The global 16x cancels downstream: exp scale absorbs 1/256 from Q16*K16,
    and the ones-column used for row-sums is set to 16 so O'/rowsum = O.
  - attn@V flipped: P^T (bf16, written directly by the ACT exp) is the
    STATIONARY operand, V (bf16, 16x) the moving one. Output lands as
    [n=128 partitions, d=64 free] — full 128-partition utilization, halving
    attn@V PE cost vs the [65, 512] layout. Row-sums via 1-cycle ones-column
    matmuls into a shared PSUM misc tile.
  - Normalization becomes a per-partition scalar in the [n, d] layout:
    reciprocal of [128, 8] gathered row-sums + tensor_scalar multiplies
    (emitted as lazy DVE fillers to avoid head-end queue bursts).
    No DRAM-bounce stride-0 broadcast, no head-10/11 special cases.
  - [n, d] -> [d, n] via XBAR DMA transposes (SBUF->SBUF bf16) on the idle
    SP queue: no PE cycles, no PSUM, no vector-engine copies. Projection
    runs in bf16 (same PE cost as f32r, half the weight DMA); k0-2 and k3
    chunks stream as fillers with DVE adds; the k4/k5 tail folds the
    accumulated k0-3 back in with an f32r identity matmul so the final
    combine is a 3-engine copy + DMA, not an add chain.
  - PSUM: psA/psB [128,1024] S double-buffer (4 banks; also reused as
    startup/tail chunk slots), two O' head tiles [128,512] (2), one filler
    tile with ping-pong 256-col halves (1), one rowsum tile (1) = 8 banks.
    start=True zeroes a whole 2KB bank, so each bank gets exactly one
    start per accumulation round (O': j0/nt0 only; rowsums: once ever),
    and only single-shot groups share banks with in-flight data.
  - GPSIMD cannot touch PSUM: Pool does only DMA issue; all PSUM reads go
    through DVE/ACT.
  - PE is the critical engine: ~114us busy (vs 135us for the unflipped
    baseline); exp stream on ACT ~100us right behind it. Cost-model span
    131.6us vs 154.9us baseline.
"""

import numpy as np

_STATE = {}

B, N, C = 8, 1024, 768
H, D = 12, 64
KT = 6           # contraction tiles of 128 over C
P = 128
NT = N // P      # 8 n-tiles
PAIRS = H // 2   # 6 head pairs
VC = 3           # V weight chunks (4 heads each)


def _patch_tile_drain():
    """This walrus build rejects >1 sem wait on a CTRL (Drain) instruction.

    TileContext's exit puts one wait per outstanding semaphore on the final SP
    Drain; redistribute them across single-wait NOPs preceding the drain.
    """
    import bass_rust
    import concourse.tile as tile
    from concourse.vector_clock import ScopedClock

    if getattr(tile.TileContext, "_ant_drain_patched", False):
        return

    SyncInfo = bass_rust.SyncInfo

    def _drain_and_barrier(self, tick_clock, wait_clock):
        nc = self.nc
        probe = nc.sync.nop(nofuse=True)
        wait_clock.add_sem_waits(
            probe.ins, ScopedClock({None: tick_clock.global_clock})
        )
        si = probe.ins.sync_info
        waits = list(si.on_wait or []) if si is not None else []
        updates = list(si.on_update or []) if si is not None else []
        if len(waits) > 1:
            probe.ins.sync_info = SyncInfo(on_wait=waits[:1], on_update=updates)
            for w in waits[1:]:
                extra = nc.sync.nop(nofuse=True)
                extra.ins.sync_info = SyncInfo(on_wait=[w], on_update=[])
        nc.sync.drain()

        nc.all_engine_barrier()
        assert self.sems is not None
        popped = nc._tile_sem_poison_stack.pop()
        assert popped is self._sem_poison
        nc.clear_and_free_semaphores(list(self.sems.allocated().values()))
        nc.all_engine_barrier()

    tile.TileContext._drain_and_barrier = _drain_and_barrier
    tile.TileContext._ant_drain_patched = True


def _split_multi_waits(nc):
    """This walrus build allows at most ONE sem wait per instruction.

    Tile's wait assignment routinely puts several; hoist all but the last onto
    single-wait NOPs inserted immediately before the instruction on the same
    engine (engines execute block instructions in order, so semantics are
    unchanged).
    """
    from concourse import mybir

    for fn in nc.m.functions:
        for bb in fn.blocks:
            out, changed = [], False
            for inst in bb.instructions:
                si = inst.sync_info
                waits = list(si.on_wait) if (si is not None and si.on_wait) else []
                if len(waits) > 1:
                    changed = True
                    for w in waits[:-1]:
                        nop = mybir.InstNoOp(
                            name=f"I-ws{nc.next_id()}",
                            engine=inst.engine,
                            bass_nofuse=True,
                            sync_info=mybir.SyncInfo(on_wait=[w], on_update=[]),
                        )
                        nc.register_instruction(nop)
                        out.append(nop)
                    inst.sync_info = mybir.SyncInfo(
                        on_wait=[waits[-1]], on_update=list(si.on_update or [])
                    )
                out.append(inst)
            if changed:
                bb.instructions = out


def _build_nc(trace_sim=False):
    from contextlib import ExitStack

    import concourse.bass as bass
    import concourse.tile as tile
    from concourse import mybir

    _patch_tile_drain()

    f32 = mybir.dt.float32
    f32r = mybir.dt.float32r
    bf16 = mybir.dt.bfloat16
    f8 = mybir.dt.float8e4

    nc = bass.Bass("TRN2", target_bir_lowering=False, debug=False, num_devices=1)

    xT = nc.dram_tensor("xT", [KT, P, 2, N], f8, kind="ExternalInput").ap()
    wqk = nc.dram_tensor("wqk", [PAIRS, P, 2 * 3 * 2 * 256], f8,
                         kind="ExternalInput").ap()
    wv = nc.dram_tensor("wv", [VC, P, 2 * 3 * 2 * 256], f8,
                        kind="ExternalInput").ap()
    ptw = nc.dram_tensor("ptw", [P, KT, C], bf16, kind="ExternalInput").ap()
    bias = nc.dram_tensor("bias", [P, C], f32, kind="ExternalInput").ap()
    ident = nc.dram_tensor("ident", [P, P], bf16, kind="ExternalInput").ap()
    ident32 = nc.dram_tensor("ident32", [P, P], f32r, kind="ExternalInput").ap()
    y = nc.dram_tensor("y", [N, C], f32, kind="ExternalOutput").ap()

    Exp = mybir.ActivationFunctionType.Exp
    DR = mybir.MatmulPerfMode.DoubleRow
    SCALE = float(D) ** -0.5 / 256.0   # /256 cancels the 16x on Q and K

    with tile.TileContext(nc, trace_sim=trace_sim) as tc, ExitStack() as ctx:
        kilo = ctx.enter_context(tc.tile_pool(name="kilo", bufs=1))     # x8
        wqkp = ctx.enter_context(tc.tile_pool(name="wqk", bufs=2))
        wvp = ctx.enter_context(tc.tile_pool(name="wv", bufs=3))
        qkp = ctx.enter_context(tc.tile_pool(name="qk", bufs=4))
        vp = ctx.enter_context(tc.tile_pool(name="v", bufs=18))
        ptp = ctx.enter_context(tc.tile_pool(name="pt", bufs=4))
        normp = ctx.enter_context(tc.tile_pool(name="nrm", bufs=18))
        rcpp = ctx.enter_context(tc.tile_pool(name="rcp", bufs=3))
        osbp = ctx.enter_context(tc.tile_pool(name="osb", bufs=24))
        outp = ctx.enter_context(tc.tile_pool(name="out", bufs=8))
        onep = ctx.enter_context(tc.tile_pool(name="one", bufs=1))
        psA = ctx.enter_context(tc.tile_pool(name="psA", bufs=1, space="PSUM"))
        psB = ctx.enter_context(tc.tile_pool(name="psB", bufs=1, space="PSUM"))
        poA = ctx.enter_context(tc.tile_pool(name="poA", bufs=1, space="PSUM"))
        poB = ctx.enter_context(tc.tile_pool(name="poB", bufs=1, space="PSUM"))
        fillp = ctx.enter_context(tc.tile_pool(name="fil", bufs=1, space="PSUM"))
        miscp = ctx.enter_context(tc.tile_pool(name="msc", bufs=1, space="PSUM"))

        # ---- persistent PSUM tiles (column-slice accumulation groups) ----
        # NOTE: a matmul with start=True marks the whole 2KB bank as
        # pending-zero, wiping any OTHER accumulation group in that bank.
        # So: misct holds ONLY the 96 one-col rowsum groups (one start ever);
        # transposes ride the fillt rotation (single-shot groups are safe:
        # completed data is read from plain memory by DVE/ACT).
        fillt = fillp.tile([P, 512], f32, tag="fill", name="fillt")
        misct = miscp.tile([P, 512], f32, tag="misc", name="misct")

        # ---- input DMAs ----
        x8 = kilo.tile([P, KT, 2, N], f8, tag="kilo", name="x8")
        wq_tiles = {}

        def prefetch_wq(t, eng):
            if t not in wq_tiles:
                wq_t = wqkp.tile([P, 2, 3, 2, 256], f8, tag="wqk", name=f"wq_{t}")
                eng.dma_start(wq_t[:], wqk[t])
                wq_tiles[t] = wq_t

        prefetch_wq(0, nc.gpsimd)  # leads the Pool queue: ready ~1.2us
        for k in range(KT):
            eng = (nc.sync, nc.scalar, nc.sync, nc.scalar,
                   nc.sync, nc.scalar)[k]
            eng.dma_start(x8[:, k, :, :], xT[k])

        # warm the ACT exp table set while input DMAs run
        warm = onep.tile([1, 4], f32)
        nc.vector.memset(warm[:], 0.0)
        warm2 = onep.tile([1, 4], f32)
        nc.scalar.activation(warm2[:], warm[:], Exp)

        # V weights chunks 0-1 early (Pool queue); 2 later
        wv_sb = [wvp.tile([P, 2, 3, 2, 256], f8, tag="wv", name=f"wvc_{c}")
                 for c in range(VC)]
        nc.gpsimd.dma_start(wv_sb[0][:], wv[0])
        nc.gpsimd.dma_start(wv_sb[1][:], wv[1])

        ident_sb = onep.tile([P, P], bf16)
        nc.sync.dma_start(ident_sb[:], ident[:])
        ident32_sb = onep.tile([P, P], f32r)
        nc.sync.dma_start(ident32_sb[:], ident32[:])
        bias_sb = onep.tile([P, C], f32)

        # ones column (=16, matching the 16x-scaled V) for row-sum matmuls
        ones_bf = onep.tile([P, 1], bf16)
        nc.vector.memset(ones_bf[:], 16.0)

        # PE p-state pre-warm: dummy matmuls while the first inputs stream in
        dm_sb = onep.tile([P, 128], bf16)
        nc.vector.memset(dm_sb[:], 0.0)
        for i in range(2):
            nc.tensor.matmul(fillt[0:1, 0:128], dm_sb[:, 0:1], dm_sb[:, 0:128],
                             start=True, stop=True, skip_group_check=True)

        # ---- QK chunks: 9 DR matmuls -> ONE psum group -> one DVE copy ----
        qt_sb, kt_sb = {}, {}
        qk_done = {}
        v_done = {}
        fill_idx = [0]

        def fill_half():
            h = fill_idx[0] % 2
            fill_idx[0] += 1
            return fillt[:, h * 256:(h + 1) * 256]

        def qk_chunk(t, which, ns, slot=None):
            store = qt_sb if which == 0 else kt_sb
            if t not in store:
                store[t] = qkp.tile([P, N], f32r, tag="qk",
                                    name=f"{'q' if which == 0 else 'k'}_{t}")
            wq_t = wq_tiles[t]
            if slot is None:
                slot = fill_half()
            ncol = slice(ns * 256, (ns + 1) * 256)
            wcol = slice(which * P, (which + 1) * P)
            passes = [(0, 0), (1, 0), (0, 1)]   # (w hi/lo, x hi/lo)
            i = 0
            for whl, xhl in passes:
                for kp in range(3):
                    nc.tensor.matmul(
                        slot, wq_t[:, whl, kp, :, wcol],
                        x8[:, 2 * kp: 2 * kp + 2, xhl, ncol],
                        start=(i == 0), stop=(i == 8), perf_mode=DR,
                        skip_group_check=True,
                    )
                    i += 1
            dest = store[t][:, ns * 256:(ns + 1) * 256]
            with nc.allow_low_precision(reason="f32r is f32 bits"):
                nc.vector.tensor_copy(dest, slot)
            qk_done[t] = qk_done.get(t, 0) + 1

        # ---- V chunks: [128 n, 256] (4 heads x 64), bf16, 16x scaled ----
        v_sb = {}

        def v_chunk(c, j):
            jcol = slice(j * P, (j + 1) * P)
            slot = fill_half()
            passes = [(0, 0), (0, 1), (1, 0)]   # (x hi/lo, w hi/lo)
            i = 0
            for xhl, whl in passes:
                for kp in range(3):
                    nc.tensor.matmul(
                        slot, x8[:, 2 * kp: 2 * kp + 2, xhl, jcol],
                        wv_sb[c][:, whl, kp, :, :],
                        start=(i == 0), stop=(i == 8), perf_mode=DR,
                        skip_group_check=True,
                    )
                    i += 1
            vt = vp.tile([P, 256], bf16, tag="v", name=f"v_{c}_{j}")
            with nc.allow_low_precision(reason="attn probs tolerate bf16 V"):
                nc.vector.tensor_copy(vt[:], slot)
            v_sb[(c, j)] = vt
            v_done[c] = v_done.get(c, 0) + 1

        # pair-0 QK immediately (chasing the input DMA arrivals). The S/O'
        # psum banks are idle at startup: give every chunk its own slot so
        # the chunks stream without ping-pong WAR stalls. S_0 needs all four
        # q chunks but only k chunk 0 — k chunks 1-3 are deferred into the
        # first head's j-loop so the exp stream starts ~3us earlier.
        st_a = psA.tile([P, N], f32, tag="psA", name="st_a")
        st_b = psB.tile([P, N], f32, tag="psB", name="st_b")
        st_slots = [st_a[:, i * 256:(i + 1) * 256] for i in range(4)] + \
                   [st_b[:, i * 256:(i + 1) * 256] for i in range(4)]
        for ns in range(4):
            qk_chunk(0, 0, ns, slot=st_slots[ns])
        for ns in range(4):
            qk_chunk(0, 1, ns, slot=st_slots[4 + ns])

        # ---- filler queue: PE work interleaved into the attention stream.
        # Priority: qk (gates the next pair's S) > v > transposes > proj.
        import heapq

        fillers = []
        fseq = [0]
        steps = [0]
        TOT_STEPS = 96

        def push_filler(prio, fn):
            heapq.heappush(fillers, (prio, fseq[0], fn))
            fseq[0] += 1

        def pop_filler(budget):
            for _ in range(budget):
                if fillers:
                    heapq.heappop(fillers)[2]()

        dve_fillers = []

        def step_fillers():
            steps[0] += 1
            left = max(1, TOT_STEPS - 8 - steps[0])
            budget = min(4, max(2, -(-len(fillers) // left)))
            pop_filler(budget)
            for _ in range(2):
                if dve_fillers:
                    dve_fillers.pop(0)()

        def ensure_qk(tp):
            # all 8 qk chunks of pair tp must be EMITTED before its first S
            while qk_done.get(tp, 0) < 8 and fillers:
                heapq.heappop(fillers)[2]()

        def ensure_v(c):
            while v_done.get(c, 0) < 8 and fillers:
                heapq.heappop(fillers)[2]()

        # ---- projection ----
        # k0-2: 256-col chunks + bias add into acc (pairs 3-4).
        # k3-4: 128-col chunks with a 4-quarter fill rotation + adds split
        #       DVE/Pool (pair 5 — add-latency-bound, so minimize WAR depth).
        # k5:   tail chunks fold acc back in via an identity matmul (f32r),
        #       then a 3-way-engine copy (ACT is free post-stream) + y DMA.
        ptw_sb = [None]
        acc_sb = {}
        q_idx = [0]
        cp_idx = [0]

        def get_acc(nt):
            if nt not in acc_sb:
                acc_sb[nt] = outp.tile([P, C], f32r, tag="out", name=f"acc_{nt}")
            return acc_sb[nt]

        def proj_g0(nt, cc):
            c0, w = cc * 256, 256
            slot = fill_half()
            for k in (0, 1, 2):
                nc.tensor.matmul(
                    slot,
                    osl(k, nt),
                    ptw_sb[0][:, k, c0:c0 + w],
                    start=(k == 0), stop=(k == 2),
                    skip_group_check=True,
                )
            dst = get_acc(nt)[:, c0:c0 + w]
            with nc.allow_low_precision(reason="f32r is f32 bits"):
                nc.vector.tensor_add(dst, slot, bias_sb[:, c0:c0 + w])

        def proj_k1(nt, cc, k):
            c0, w = cc * 256, 256
            slot = fill_half()
            nc.tensor.matmul(
                slot,
                osl(k, nt),
                ptw_sb[0][:, k, c0:c0 + w],
                start=True, stop=True,
                skip_group_check=True,
            )
            dst = get_acc(nt)[:, c0:c0 + w]
            with nc.allow_low_precision(reason="f32r is f32 bits"):
                nc.vector.tensor_add(dst, dst, slot)

        tail_slots = [None]

        def proj_tail(nt):
            # 3 cc-chunks into CONSECUTIVE slices of one S-bank tile, then
            # ONE wide [128,768] copy + one y DMA per nt: 8 copies instead
            # of 24 keeps the tail PE-bound instead of copy-bound
            base = tail_slots[0][nt % 2]
            acc = get_acc(nt)
            for cc in range(3):
                c0, w = cc * 256, 256
                slot = base[:, c0:c0 + w]
                for k in (3, 4, 5):
                    nc.tensor.matmul(
                        slot, osl(k, nt),
                        ptw_sb[0][:, k, c0:c0 + w],
                        start=(k == 3), stop=False, skip_group_check=True,
                    )
                nc.tensor.matmul(
                    slot, ident32_sb[:],
                    acc[:, c0:c0 + w],
                    start=False, stop=True, skip_group_check=True,
                )
            eng = (nc.scalar, nc.vector)[nt % 2]
            with nc.allow_low_precision(reason="f32r is f32 bits"):
                if eng is nc.scalar:
                    eng.copy(acc[:], base[:, 0:C])
                else:
                    eng.tensor_copy(acc[:], base[:, 0:C])
            deng = nc.sync if nt % 2 == 0 else nc.scalar
            deng.dma_start(y[nt * P:(nt + 1) * P, :], acc[:].bitcast(f32))

        # ---- attention ----
        o_sb = {}          # (pair, np2) -> [128 d-pair, 256] bf16
        # per-np2 tiles: DMA-transpose writes are tile-granular for dep
        # tracking, so readers of one n-tile must not wait all 8 transposes

        def osl(k, nt):
            return o_sb[(k, nt // 2)][:, (nt % 2) * P:(nt % 2 + 1) * P]
        norm_pair = {}     # pair -> [128 n, 8*(2*64)] bf16 (normalized O)
        tr_idx = [0]

        def emit_S(t, head, j):
            hb = head * D
            pool, tg = (psA, "psA") if j % 2 == 0 else (psB, "psB")
            s_slot = pool.tile([P, N], f32, tag=tg, name=f"s_{2*t+head}_{j}")
            for ns in range(2):
                nc.tensor.matmul(
                    s_slot[:, ns * 512:(ns + 1) * 512],
                    kt_sb[t][hb:hb + D, j * P:(j + 1) * P],
                    qt_sb[t][hb:hb + D, ns * 512:(ns + 1) * 512],
                    start=True, stop=True,
                )
            return s_slot

        norms_done = {}

        def transpose_chunk(t, np2):
            # [n, d-pair] -> [d-pair, n] via XBAR DMA transposes on the idle
            # SP queue: no PE cycles, no PSUM, no DVE copies.
            while norms_done.get(t, 0) < 16 and dve_fillers:
                dve_fillers.pop(0)()
            for i in range(2):
                nt = 2 * np2 + i
                nc.sync.dma_start_transpose(
                    osl(t, nt),
                    norm_pair[t][:, nt * P:(nt + 1) * P])

        pre_s = [[]]
        rs_started = [False]
        for t in range(PAIRS):
            for np2 in range(NT // 2):
                o_sb[(t, np2)] = osbp.tile(
                    [P, 256], bf16, tag="osb", name=f"ot_{t}_{np2}")
            if t < PAIRS - 1:
                prefetch_wq(t + 1, nc.gpsimd if t == 0 else nc.sync)
                for which in range(2):
                    for ns in range(4):
                        push_filler(
                            0, lambda t=t, w=which, n=ns: qk_chunk(t + 1, w, n))
            # v chunks: all of c=1 at pair 0; c=2 split over pairs 2-3
            v_push = {0: (1, range(NT)), 2: (2, range(0, 4)),
                      3: (2, range(4, NT))}
            if t in v_push:
                c, js = v_push[t]
                for j in js:
                    push_filler(1, lambda c=c, j=j: v_chunk(c, j))
            if t == 0:
                nc.sync.dma_start(wv_sb[2][:], wv[2])
            if t == 1:
                ptw_t = onep.tile([P, KT, C], bf16, name="ptw_sb")
                nc.gpsimd.dma_start(ptw_t[:], ptw[:])
                ptw_sb[0] = ptw_t
                nc.sync.dma_start(bias_sb[:], bias[:])
            if 1 <= t <= 5:
                # transposes for pair t-1 (norm tiles ready at end of pair t-1)
                for np2 in range(NT // 2):
                    push_filler(2, lambda tt=t - 1, np2=np2:
                                transpose_chunk(tt, np2))
            if t == 3:
                # k0-2 proj (pairs 0-2 transposed by pair 3's tr batch);
                # nt 0-3 here, nt 4-7 at pair 4
                for nt in range(4):
                    for cc in range(3):
                        push_filler(3, lambda nt=nt, cc=cc: proj_g0(nt, cc))
            if t == 4:
                for nt in range(4, NT):
                    for cc in range(3):
                        push_filler(3, lambda nt=nt, cc=cc: proj_g0(nt, cc))


            if t > 0:
                ensure_v(t // 2)
            for head in range(2):
                h = 2 * t + head
                opool, otag = (poA, "poA") if head == 0 else (poB, "poB")
                o_slot = opool.tile([P, 512], f32, tag=otag, name=f"o_{h}")
                vc = t // 2
                vb = h - 4 * vc  # head index within the 4-head V chunk
                nxt = (t + (head + 1) // 2, 1 - head) if h < 11 else None

                def emit_exp(j, s_slot):
                    # emitted immediately after its S matmuls so the exp's
                    # PE-clock wait covers ONLY the S
                    pt_t = ptp.tile([P, N], bf16, tag="pt", name=f"p_{h}_{j}")
                    nc.scalar.activation(pt_t[:], s_slot[:], Exp, scale=SCALE)
                    return pt_t

                if pre_s[0]:
                    s0, s1 = pre_s[0]
                else:
                    s0, s1 = emit_S(t, head, 0), emit_S(t, head, 1)
                pre_s[0] = []
                pts = [emit_exp(0, s0), emit_exp(1, s1)]
                if t == 0 and head == 0:
                    v_chunk(0, 0)
                    v_chunk(0, 1)
                for j in range(NT):
                    if j + 2 < NT:
                        pts.append(emit_exp(j + 2, emit_S(t, head, j + 2)))
                    elif nxt is not None:
                        # pre-emit the next head's S_0/S_1 so its exp stream
                        # starts without a boundary stall
                        ensure_qk(nxt[0])
                        pre_s[0].append(emit_S(nxt[0], nxt[1], j - 6))
                    vtile = v_sb[(vc, j)]
                    for nt in range(NT):
                        # start=True zeroes the whole bank: exactly ONE start
                        # per head (j0/nt0); later nt's j0 writes land on the
                        # pending-zero region = fresh accumulation
                        nc.tensor.matmul(
                            o_slot[:, nt * 64:(nt + 1) * 64],
                            pts[j][:, nt * P:(nt + 1) * P],
                            vtile[:, vb * D:(vb + 1) * D],
                            start=(j == 0 and nt == 0), stop=(j == NT - 1),
                            skip_group_check=True,
                        )
                        # rowsums share one bank across ALL heads: start once
                        nc.tensor.matmul(
                            misct[:, h * 8 + nt: h * 8 + nt + 1],
                            pts[j][:, nt * P:(nt + 1) * P],
                            ones_bf[:],
                            start=(not rs_started[0]), stop=(j == NT - 1),
                            skip_group_check=True,
                        )
                        rs_started[0] = True
                    if t == 0 and head == 0:
                        if j + 2 < NT:
                            v_chunk(0, j + 2)
                        steps[0] += 1
                    else:
                        step_fillers()
                # head end: reciprocal of the 8 gathered row-sums, then
                # per-partition normalize into the pair's norm tiles
                rc_t = rcpp.tile([P, NT], f32, tag="rcp", name=f"rc_{h}")
                nc.vector.reciprocal(rc_t[:], misct[:, h * 8:(h + 1) * 8])

                if t not in norm_pair:
                    norm_pair[t] = normp.tile(
                        [P, N], bf16, tag="nrm", name=f"nm_{t}")

                def norm_all(t=t, head=head, o_slot=o_slot, rc_t=rc_t):
                    # ONE strided DVE multiply for the whole head: o_slot
                    # [128, 8x64] x rc broadcast (stride-0 inner dim) ->
                    # norm_pair cols nt*128 + head*64
                    out = norm_pair[t].rearrange(
                        "p (nt two d) -> p nt two d", two=2, d=D)[:, :, head, :]
                    src = o_slot.rearrange("p (nt d) -> p nt d", d=D)
                    rcb = bass.AP(
                        tensor=rc_t.tensor,
                        offset=rc_t.offset,
                        ap=[list(rc_t[:, :].ap[0]), [1, NT], [0, D]],
                    )
                    with nc.allow_low_precision(reason="bf16 attn out"):
                        nc.vector.tensor_mul(out, src, rcb)
                    norms_done[t] = norms_done.get(t, 0) + 8

                if h < 11:
                    # lazily popped: avoids head-end DVE bursts blocking
                    # the chunk-copy stream
                    dve_fillers.append(norm_all)
                else:
                    norm11 = norm_all

        # ---- tail: pair-5 transposes + k4/k5 projection + output. The S
        # psum banks are free now: deep slot rotation so nothing ping-pong
        # stalls; leftover fillers interleave to hide the per-nt chain.
        st2a = psA.tile([P, N], f32, tag="psA", name="st2a")
        st2b = psB.tile([P, N], f32, tag="psB", name="st2b")
        tail_slots[0] = (st2a, st2b)
        while dve_fillers:
            dve_fillers.pop(0)()
        norm11()
        pop_filler(len(fillers))
        for np2 in range(NT // 2):
            transpose_chunk(5, np2)
            proj_tail(2 * np2)
            proj_tail(2 * np2 + 1)

    _split_multi_waits(nc)
    return nc


def _prep_shared(qkv_w, proj_w, proj_b):
    import ml_dtypes

    E4 = ml_dtypes.float8_e4m3fn
    BF = ml_dtypes.bfloat16
    f = np.float32

    def dsplit(a):
        hi = a.astype(E4)
        lo = (a - hi.astype(f)).astype(E4)
        return hi, lo

    wqT = np.ascontiguousarray(qkv_w[0:C].T.astype(f)) * 16.0      # [c, o]
    wkT = np.ascontiguousarray(qkv_w[C:2 * C].T.astype(f)) * 16.0
    wvT = np.ascontiguousarray(qkv_w[2 * C:3 * C].T.astype(f)) * 16.0

    wqk = np.zeros((PAIRS, P, 2, 3, 2, 256), E4)
    for t in range(PAIRS):
        for kp in range(3):
            for pe in range(2):  # k-tile within the DoubleRow pair
                k = 2 * kp + pe
                qh, ql = dsplit(wqT[k * P:(k + 1) * P, t * P:(t + 1) * P])
                kh, kl = dsplit(wkT[k * P:(k + 1) * P, t * P:(t + 1) * P])
                wqk[t, :, 0, kp, pe, 0:P] = qh
                wqk[t, :, 1, kp, pe, 0:P] = ql
                wqk[t, :, 0, kp, pe, P:256] = kh
                wqk[t, :, 1, kp, pe, P:256] = kl
    wqk = wqk.reshape(PAIRS, P, 2 * 3 * 2 * 256)

    wvh = np.zeros((VC, P, 2, 3, 2, 256), E4)
    for c in range(VC):
        for kp in range(3):
            for pe in range(2):
                k = 2 * kp + pe
                vh, vl = dsplit(
                    wvT[k * P:(k + 1) * P, 4 * c * D:(4 * c + 4) * D])
                wvh[c, :, 0, kp, pe, :] = vh
                wvh[c, :, 1, kp, pe, :] = vl
    wvh = wvh.reshape(VC, P, 2 * 3 * 2 * 256)

    ptw = proj_w.T.astype(f).reshape(KT, P, C).transpose(1, 0, 2)
    ptw = np.ascontiguousarray(ptw).astype(BF)
    bias_h = np.ascontiguousarray(np.broadcast_to(proj_b.astype(f), (P, C)))
    ident = np.eye(P, dtype=BF)
    ident32 = np.eye(P, dtype=f)
    return wqk, wvh, ptw, bias_h, ident, ident32


def kernel(x, qkv_w, proj_w, proj_b):
    import ml_dtypes
    from concourse.bass_utils import run_bass_kernel_spmd

    E4 = ml_dtypes.float8_e4m3fn
    x = np.asarray(x, np.float32)
    wqk, wvh, ptw, bias_h, ident, ident32 = _prep_shared(
        np.asarray(qkv_w), np.asarray(proj_w), np.asarray(proj_b)
    )

    if "nc" not in _STATE:
        _STATE["nc"] = _build_nc()
    nc = _STATE["nc"]

    in_maps = []
    for b in range(B):
        xf = np.ascontiguousarray(x[b].T).reshape(KT, P, N)
        xh = xf.astype(E4)
        xl = (xf - xh.astype(np.float32)).astype(E4)
        xTb = np.stack([xh, xl], axis=2)  # [KT, P, 2, N] fp8
        in_maps.append(
            {"xT": xTb, "wqk": wqk, "wv": wvh, "ptw": ptw, "bias": bias_h,
             "ident": ident, "ident32": ident32}
        )

    res = run_bass_kernel_spmd(nc, in_maps, core_ids=list(range(B)))
    return np.stack([res.results[b]["y"] for b in range(B)], axis=0)
